# revision 26
# baseline (speedup 1.0000x reference)
"""CBTree bottom-up fold kernel for 8 trn2 NeuronCores.

Problem: complete 4-ary tree, 9 levels, 87381 nodes in BFS order, d=256.
  leaves (level 8): h = vectors[21845:]
  internal node:    h = tanh(sum_i W_i @ h_child_i + vectors[node])
  where W_i = lc[i]*Wl + rc[i]*Wr,  lc=[1,2/3,1/3,0], rc=[0,1/3,2/3,1].

Strategy (data-parallel over sibling groups):
  - Shard every level contiguously over 8 cores. Children of a core's
    parents are exactly the core's own previous-level outputs, so
    levels 7..2 run with zero communication (level-2 node j's children
    [4j,4j+4) lie inside core j//2's level-3 slice [8r,8r+8)).
  - One tiny AllGather of the level-2 states (16 nodes), then every
    core redundantly folds levels 1..0 and writes the root.
  - On chip h lives transposed ([d, nodes], two 128-row partition
    halves) so the tensor engine contracts over d; the host hands each
    core its slices already in this layout.
  - All stream DMAs are issued up-front in consumption order on the SP
    queue (in-order, self-pacing); level-6 compute chunks are emitted
    interleaved into level-7's DMA-bound bubbles to keep PE busy.
  - fp16 everywhere on chip (fp32 PSUM accumulation): ~3.4e-3
    scale-relative error. fp8/bf16 blow the 2e-2 budget (error
    amplifies ~20x through the 8-level fold).
  - The root would be an N=1 matmul (invalid ISA), so the last level
    computes 4 replicated root copies; the output DMA reads copy 0
    with a strided AP straight into the [1,256] fp16 out tensor (host
    upcasts to f32).
"""

import numpy as np

F16 = None  # set on first _lazy_imports()

_BASS = {}


def _lazy_imports():
    global bass, bacc, mybir, tile, run_bass_kernel_spmd, F16, F32
    import concourse.bass as bass
    import concourse.mybir as mybir
    from concourse import bacc
    import concourse.tile as tile
    from concourse.bass_utils import run_bass_kernel_spmd
    F16 = mybir.dt.float16
    F32 = mybir.dt.float32

N_CORES = 8
D = 256
B = 4
L = 9
SIZES = [B**l for l in range(L)]            # [1,4,16,64,256,1024,4096,16384,65536]
OFFSETS = np.concatenate([[0], np.cumsum(SIZES)])
N_LEAF_CORE = SIZES[8] // N_CORES           # 8192
LOC_LEVELS = [7, 6, 5, 4, 3, 2]
LOC_PAR = {l: SIZES[l] // N_CORES for l in LOC_LEVELS}  # 2048,512,128,32,8,2
N_VECS_LOC = sum(LOC_PAR.values())          # 2730
N_VECS_TAIL = 4 + SIZES[1]                  # 4x replicated root + 4 L1 nodes
L7_CHUNKS = [128, 256, 512, 512, 512, 128]
L6_CHUNKS = [128, 128, 128, 128]
# filler matmuls (PE-warm) emitted after selected L7 chunks: chunk idx -> count
FILLERS = {}
WARMS = []


def _build_nc(mode="fp16", warms=None, fillers=None, wsb_order="early",
              plan=None, pbufs=6):
    if warms is None:
        warms = WARMS
    if fillers is None:
        fillers = FILLERS
    key = ("nc", mode, tuple(warms), tuple(sorted(fillers.items())), wsb_order,
           tuple(plan) if plan else None, pbufs)
    if key in _BASS:
        return _BASS[key]
    assert mode == "fp16"
    nc = bacc.Bacc(num_devices=N_CORES)

    leavesT = nc.declare_dram_parameter("leavesT", [D, N_LEAF_CORE], F16, isOutput=False)
    vecs_locT = nc.declare_dram_parameter("vecs_locT", [D, N_VECS_LOC], F16, isOutput=False)
    vecs_tailT = nc.declare_dram_parameter("vecs_tailT", [D, N_VECS_TAIL], F16, isOutput=False)
    wmat = nc.declare_dram_parameter("wmat", [128, 17 * 128], F16, isOutput=False)
    out = nc.declare_dram_parameter("out", [1, D], F16, isOutput=True)

    with tile.TileContext(nc) as tc:
        with (
            tc.tile_pool(name="const", bufs=1) as const_pool,
            tc.tile_pool(name="hbuf", bufs=1) as hbuf,
            tc.tile_pool(name="vecp", bufs=1) as vec_pool,
            tc.tile_pool(name="pmm", bufs=pbufs, space="PSUM") as psum_mm,
            tc.tile_pool(name="pwarm", bufs=1, space="PSUM") as psum_warm,
            tc.tile_pool(name="dram", bufs=1, space="DRAM") as dram_pool,
        ):
            # weight blocks: mh=0 blocks (kh,i)=0..7, identity at 8, mh=1 at 9..16
            wsb = const_pool.tile([128, 17 * 128], F16, name="wsb")

            # activation-table warm on a zeroed tile
            warm = const_pool.tile([128, 8], F32, name="warm")
            nc.gpsimd.memset(warm[:], 0.0)
            nc.scalar.activation(warm[:1, :4], warm[:1, :4],
                                 mybir.ActivationFunctionType.Tanh)

            # PE p-state pre-ramp: fp32 warm matmuls (4 cyc/row) starting as
            # soon as the zeroed tile exists, abutting the first real chunk
            def pe_warm_f32(n):
                wps = psum_warm.tile([128, 512], F32, name="ps_w", tag="w")
                nc.tensor.matmul(wps[:8, :n], warm[:, 0:8],
                                 warm[:, 0:1].broadcast_to([128, n]),
                                 start=True, stop=True)

            def pe_fill_f16(n):
                wps = psum_warm.tile([128, 512], F32, name="ps_w", tag="w")
                nc.tensor.matmul(wps[:, :n], wsb[:, 0:128], wsb[:, 0:n],
                                 start=True, stop=True)

            def h_tiles(name, n):
                return [hbuf.tile([128, max(n, 1)], F16, name=f"{name}_{kh}",
                                  tag=f"{name}_{kh}") for kh in (0, 1)]

            hT8 = h_tiles("hT8", N_LEAF_CORE)
            hT = {7: h_tiles("hT7", 2048), 6: h_tiles("hT6", 512),
                  5: h_tiles("hT5", 128), 4: h_tiles("hT4", 32),
                  3: h_tiles("hT3", 8)}
            NL2 = LOC_PAR[2]                                # 2
            t2p = hbuf.tile([128, 2, NL2], F16, name="hT2p", tag="hT2p")
            hT[2] = [t2p[:, 0, :], t2p[:, 1, :]]
            # gathered level-2 states, columns ordered (r, kh, n) so the
            # unbounce is ONE 3D DMA; level-1 rhs uses stride-8 column APs
            h2g = hbuf.tile([128, 2 * SIZES[2]], F16, name="h2g", tag="h2g")
            t1p = hbuf.tile([128, 2, SIZES[1]], F16, name="hT1p", tag="hT1p")
            t0p = hbuf.tile([128, 2, 4], F16, name="hT0p", tag="hT0p")

            vloc = vec_pool.tile([128, 2, N_VECS_LOC], F16, name="vloc", tag="vloc")
            vtail = vec_pool.tile([128, 2, N_VECS_TAIL], F16, name="vtail", tag="vtail")

            def vec_dma(col0, n):
                nc.sync.dma_start(
                    vloc[:, :, col0:col0 + n],
                    vecs_locT[:, col0:col0 + n].rearrange("(mh k) n -> k mh n", mh=2))

            def leaf_dma(col0, n):
                for kh in (0, 1):
                    nc.sync.dma_start(
                        hT8[kh][:, col0:col0 + n],
                        leavesT[kh * 128:(kh + 1) * 128, col0:col0 + n])

            # ---- the PE pre-ramp, then the input stream in consumption
            # order on SP (in-order queue self-paces) ----
            for n in warms:
                pe_warm_f32(n)
            bounds7 = np.concatenate([[0], np.cumsum(L7_CHUNKS)])
            assert bounds7[-1] == LOC_PAR[7]

            def lf(k):
                return lambda: leaf_dma(4 * int(bounds7[k]), 4 * L7_CHUNKS[k])

            def vc(k):
                return lambda: vec_dma(int(bounds7[k]), L7_CHUNKS[k])

            wsbA = lambda: nc.sync.dma_start(wsb[:, :9 * 128], wmat[:, :9 * 128])
            wsbB = lambda: nc.sync.dma_start(wsb[:, 9 * 128:], wmat[:, 9 * 128:])
            vrest = lambda: vec_dma(LOC_PAR[7], N_VECS_LOC - LOC_PAR[7])
            vt = lambda: nc.sync.dma_start(
                vtail[:], vecs_tailT[:].rearrange("(mh k) n -> k mh n", mh=2))
            vec6a = lambda: vec_dma(LOC_PAR[7], LOC_PAR[6])
            vrest2 = lambda: vec_dma(LOC_PAR[7] + LOC_PAR[6],
                                     N_VECS_LOC - LOC_PAR[7] - LOC_PAR[6])
            if wsb_order == "early":
                stream = [wsbA, wsbB, lf(0), vc(0), lf(1), vc(1), lf(2), vc(2),
                          lf(3), vec6a, vc(3), lf(4), vc(4), lf(5), vc(5),
                          vrest2, vt]
            elif wsb_order == "split":
                stream = [wsbA, lf(0), vc(0), wsbB, lf(1), vc(1), lf(2), vc(2),
                          lf(3), vec6a, vc(3), lf(4), vc(4), lf(5), vc(5),
                          vrest2, vt]
            else:  # leaf0 first
                stream = [lf(0), wsbA, vc(0), wsbB, lf(1), vc(1), lf(2), vc(2),
                          lf(3), vec6a, vc(3), lf(4), vc(4), lf(5), vc(5),
                          vrest2, vt]
            for dma in stream:
                dma()

            # ---- compute: one (level, chunk) psum-group pair ----
            def do_chunk(rview, c0, N, vec_tile, vec_col0, hT_out):
                for mh in (0, 1):
                    ps = psum_mm.tile([128, 512], F32, name="ps_mm", tag="mm")
                    for kh in (0, 1):
                        for i in range(4):
                            blk = (9 if mh else 0) + kh * 4 + i
                            w = wsb[:, blk * 128:(blk + 1) * 128]
                            rhs = rview[kh][:, c0:c0 + N, i]
                            nc.tensor.matmul(ps[:, :N], w, rhs,
                                             start=(i == 0 and kh == 0),
                                             stop=False)
                    nc.tensor.matmul(
                        ps[:, :N], wsb[:, 8 * 128:9 * 128],
                        vec_tile[:, mh, vec_col0 + c0: vec_col0 + c0 + N],
                        start=False, stop=True)
                    nc.scalar.activation(hT_out[mh][:, c0:c0 + N], ps[:, :N],
                                         mybir.ActivationFunctionType.Tanh)

            def rv(child, n_par):
                return [child[kh][:, :4 * n_par].rearrange(
                    "k (p four) -> k p four", four=4) for kh in (0, 1)]

            rview = {7: rv(hT8, LOC_PAR[7]), 6: rv(hT[7], LOC_PAR[6]),
                     5: rv(hT[6], LOC_PAR[5]), 4: rv(hT[5], LOC_PAR[4]),
                     3: rv(hT[4], LOC_PAR[3]), 2: rv(hT[3], LOC_PAR[2])}
            vcol = {}
            acc = 0
            for l in LOC_LEVELS:
                vcol[l] = acc
                acc += LOC_PAR[l]
            # L6 chunks interleave into L7's DMA-bound bubbles; levels 5..2
            # run as single chunks (strip-cascading them measured slower:
            # each extra strip adds an act handoff that outweighs the
            # shorter final chain)
            if plan is None:
                plan = [(7, 0, 128), (7, 128, 256), (7, 384, 512), (6, 0, 128),
                        (7, 896, 512), (6, 128, 128), (7, 1408, 512),
                        (6, 256, 128), (7, 1920, 128), (6, 384, 128),
                        (5, 0, 128), (4, 0, 32), (3, 0, 8), (2, 0, 2)]
            covered = {l: 0 for l in LOC_LEVELS}
            for lvl, c0, n in plan:
                assert c0 == covered[lvl], (lvl, c0)
                if lvl < 7:  # children must be ready
                    assert 4 * (c0 + n) <= covered[lvl + 1], (lvl, c0, n)
                covered[lvl] = c0 + n
                do_chunk(rview[lvl], c0, n, vloc, vcol[lvl], hT[lvl])
            assert all(covered[l] == LOC_PAR[l] for l in LOC_LEVELS)

            # ---- AllGather of level-2 states ----
            cc_in = dram_pool.tile([D, NL2], F16, name="cc_in")
            cc_out = dram_pool.tile([N_CORES * D, NL2], F16, name="cc_out")
            nc.sync.dma_start(
                cc_in[:].rearrange("(kh k) n -> k kh n", kh=2), t2p[:])
            nc.gpsimd.collective_compute(
                "AllGather", mybir.AluOpType.bypass,
                replica_groups=[list(range(N_CORES))],
                ins=[cc_in.opt()], outs=[cc_out.opt()])
            # gathered rows are (q=(r,kh), k); one 3D DMA into columns (q, n)
            nc.sync.dma_start(
                h2g[:].rearrange("k (q n) -> k q n", n=NL2),
                cc_out[:].rearrange("(q k) n -> k q n", k=128))

            # ---- tail: level 1 (4 parents), then 4 root copies ----
            # L2 node m=4j+i lives at column 8j + 4*(i//2) + 2*kh + i%2
            h2r = h2g[:].rearrange("k (j e) -> k j e", e=8)
            rview1 = None  # custom per-(i,kh) columns

            def do_tail(rhs_fn, N, vec_col0, out_tile):
                for mh in (0, 1):
                    ps = psum_mm.tile([128, 512], F32, name="ps_mm", tag="mm")
                    for kh in (0, 1):
                        for i in range(4):
                            blk = (9 if mh else 0) + kh * 4 + i
                            w = wsb[:, blk * 128:(blk + 1) * 128]
                            nc.tensor.matmul(ps[:, :N], w, rhs_fn(i, kh),
                                             start=(i == 0 and kh == 0),
                                             stop=False)
                    nc.tensor.matmul(
                        ps[:, :N], wsb[:, 8 * 128:9 * 128],
                        vtail[:, mh, vec_col0:vec_col0 + N],
                        start=False, stop=True)
                    nc.scalar.activation(out_tile[:, mh, :N], ps[:, :N],
                                         mybir.ActivationFunctionType.Tanh)

            do_tail(lambda i, kh: h2r[:, :, 4 * (i // 2) + 2 * kh + (i % 2)],
                    SIZES[1], 4, t1p)
            do_tail(lambda i, kh: t1p[:, kh, i].unsqueeze(1).broadcast_to([128, 4]),
                    4, 0, t0p)

            # ---- write the root: strided fp16 DMA, no transpose ----
            nc.sync.dma_start(
                out[:].rearrange("o (kh k) -> k o kh", kh=2),
                t0p[:, :, 0:1].rearrange("k kh o -> k o kh"))

    nc.finalize()
    _BASS[key] = nc
    return nc


def _prep_inputs(vectors, Wl, Wr):
    vectors = np.asarray(vectors, dtype=np.float32)
    Wl = np.asarray(Wl, dtype=np.float32)
    Wr = np.asarray(Wr, dtype=np.float32)

    ind = np.arange(1, B + 1, dtype=np.float32)
    lc = (B - ind) / (B - 1)
    rc = (ind - 1) / (B - 1)
    # W_t[i] = W_i.T; block order mh0(8) | identity | mh1(8)
    Wt = np.stack([lc[i] * Wl.T + rc[i] * Wr.T for i in range(B)])  # [4, 256k, 256m]
    W5 = Wt.reshape(4, 2, 128, 2, 128)            # [i, kh, k', mh, m']
    halves = [W5[:, :, :, mh, :].reshape(4, 2, 128, 128)
              .transpose(2, 1, 0, 3).reshape(128, 8 * 128) for mh in (0, 1)]
    wmat = np.ascontiguousarray(
        np.concatenate([halves[0], np.eye(128, dtype=np.float32), halves[1]],
                       axis=1), dtype=np.float32)

    vecsT = np.ascontiguousarray(vectors.T)                      # [256, 87381]
    vecs_tailT = np.ascontiguousarray(
        np.concatenate([np.repeat(vecsT[:, 0:1], 4, axis=1),
                        vecsT[:, 1:5]], axis=1))
    hdt = np.float16
    in_maps = []
    for c in range(N_CORES):
        o8 = int(OFFSETS[8])
        leavesT_c = vecsT[:, o8 + c * N_LEAF_CORE: o8 + (c + 1) * N_LEAF_CORE]
        loc_parts = []
        for l in LOC_LEVELS:
            npl = LOC_PAR[l]
            o = int(OFFSETS[l])
            loc_parts.append(vecsT[:, o + c * npl: o + (c + 1) * npl])
        im = {
            "leavesT": np.ascontiguousarray(leavesT_c).astype(hdt),
            "vecs_locT": np.ascontiguousarray(
                np.concatenate(loc_parts, axis=1)).astype(hdt),
            "vecs_tailT": vecs_tailT.astype(hdt),
            "wmat": wmat.astype(hdt),
        }
        in_maps.append(im)
    return in_maps


def kernel(vectors, Wl, Wr, branching, n_levels, _mode="fp16"):
    _lazy_imports()
    assert int(branching) == B and int(n_levels) == L
    vectors = np.asarray(vectors)
    assert vectors.shape == (int(OFFSETS[L]), D), vectors.shape

    nc = _build_nc(mode=_mode)
    in_maps = _prep_inputs(vectors, Wl, Wr)
    try:
        res = run_bass_kernel_spmd(nc, in_maps, core_ids=list(range(N_CORES)),
                                   trace=False)
    except Exception:
        # transient device hiccups clear on a retry
        res = run_bass_kernel_spmd(nc, in_maps, core_ids=list(range(N_CORES)),
                                   trace=False)
    root = res.results[0]["out"]
    return np.asarray(root, dtype=np.float32).reshape(1, D)
